# revision 1
# baseline (speedup 1.0000x reference)
"""Trainium2 Bass kernel for per-node rank-1 self-attention (NodeFeatureSelfAttention).

Math: for each node n (row of x):
    q = s*(Wq @ xp + bq); k = Wk @ xp + bk; v = Wv @ xp + bv   (xp = x + pe)
    out[i] = sum_j softmax_j(q_i * k_j)[j] * v_j = g(q_i)
with g(t) = sum_j exp(t*k_j)*v_j / sum_j exp(t*k_j), a smooth scalar function
per node. We sample g at M shared Chebyshev points t_m (ACT engine exps),
reduce with masked-ones matmuls on the PE, convert samples -> monomial
coefficients with a shared M x M matrix (PE), and evaluate the degree-(M-1)
interpolant per element with fused scalar_tensor_tensor Horner steps (DVE).

Data-parallel over nodes across 8 NeuronCores; weights replicated.
"""
import sys
sys.path.insert(0, "/opt/trn_rl_repo")
import numpy as np
from contextlib import ExitStack

N, D = 16384, 128
NCORES = 8
NLOC = N // NCORES            # 2048 nodes per core
NT = NLOC // 128              # 16 node-tiles per core
M = 9                         # Chebyshev sample count (degree M-1 interpolant)

_built = {}


DEBUG = False
EV_BF16 = False  # masks live in the f32 const blob; bf16 path needs separate masks


def _build():
    """Build + finalize the (data-independent) bass module once."""
    if "nc" in _built:
        return _built["nc"]
    import concourse.bacc as bacc
    import concourse.tile as tile
    from concourse import mybir

    f32 = mybir.dt.float32
    evdt = mybir.dt.bfloat16 if EV_BF16 else f32
    nc = bacc.Bacc()

    xs = nc.declare_dram_parameter("xs", [NLOC, D], f32, isOutput=False)
    # all constants packed into one [D, NCONST] f32 blob (one DMA):
    # cols: WQT D | WKT D | WVT D | BQB D | IDN D | FMASK 8D | AINVT4 4M | TMS M | MASKS 32M | BIASCOL 2 | bias-row D (on partition 0)
    NCONST = 5 * D + 8 * D + 4 * M + M + 32 * M + 2 + D
    CONSTS = nc.declare_dram_parameter("CONSTS", [D, NCONST], f32, isOutput=False)
    OUT = nc.declare_dram_parameter("out", [NLOC, D], f32, isOutput=True)
    if DEBUG:
        DQ = nc.declare_dram_parameter("dbg_q", [D, NLOC], f32, isOutput=True)
        DKV = nc.declare_dram_parameter("dbg_kvt", [D, 2 * NLOC], f32, isOutput=True)
        DCOEF = nc.declare_dram_parameter("dbg_coef", [M, 2 * NLOC], f32, isOutput=True)
        DG = nc.declare_dram_parameter("dbg_g", [M, NLOC], f32, isOutput=True)
        DCT = nc.declare_dram_parameter("dbg_ct", [M, NLOC], f32, isOutput=True)
        DCTS = nc.declare_dram_parameter("dbg_cts", [D, NT * M], f32, isOutput=True)

    with tile.TileContext(nc) as tc, ExitStack() as ctx:
        from concourse.mybir import AluOpType
        singles = ctx.enter_context(tc.tile_pool(name="singles", bufs=1))
        xin = ctx.enter_context(tc.tile_pool(name="xin", bufs=4))
        emp = ctx.enter_context(tc.tile_pool(name="emp", bufs=6))
        evp = ctx.enter_context(tc.tile_pool(name="evp", bufs=4))
        hor = ctx.enter_context(tc.tile_pool(name="hor", bufs=4))
        outp = ctx.enter_context(tc.tile_pool(name="outp", bufs=4))

        # ---- constants: one blob, 4 parallel DMA chunks ----
        cblob = singles.tile([D, NCONST], f32)
        ccut = [0, 2 * D, 4 * D, 9 * D, NCONST]
        for ci in range(4):
            nc.sync.dma_start(out=cblob[:, ccut[ci]:ccut[ci + 1]],
                              in_=CONSTS[:, ccut[ci]:ccut[ci + 1]])
        o = 0
        wqt = cblob[:, o:o + D]; o += D
        wkt = cblob[:, o:o + D]; o += D
        wvt = cblob[:, o:o + D]; o += D
        bqb = cblob[:, o:o + D]; o += D
        idn = cblob[:, o:o + D]; o += D
        fmask = cblob[:, o:o + 8 * D].rearrange("p (i c) -> p i c", i=8); o += 8 * D
        ainvt4 = cblob[:, o:o + 4 * M].rearrange("p (i c) -> p i c", i=4); o += 4 * M
        tms = cblob[:, o:o + M]; o += M
        masks_f = cblob[:, o:o + 32 * M].rearrange("p (i c) -> p i c", i=M); o += 32 * M
        masks = masks_f
        biascol = cblob[:, o:o + 2]; o += 2
        bias = cblob[0:1, o:o + D]  # bias-row on partition 0
        o += D

        xT_all = singles.tile([D, NT, 128], f32)      # x^T per tile
        q_all = singles.tile([D, NLOC], f32)          # Q' blocked [node_p, (t i)]
        kvt = singles.tile([D, 2, NLOC], f32)         # [j, {K^T,-}, n]
        vt_b = singles.tile([D, NLOC], evdt)          # V^T (bf16 when EV_BF16)
        cts = singles.tile([D, NT, M], f32)           # per-tile monomial coeffs
        coef_sb = singles.tile([D, 2, NLOC], f32)     # [p, {num,den}, n]
        rden = singles.tile([D, NLOC], f32)
        g_sb = singles.tile([D, NLOC], f32)
        ct_sb = singles.tile([M, NLOC], f32)

        # ---- Phase A: load x (4 DMAs), transpose + QKV staggered by one tile ----
        x_sb = singles.tile([D, NT, D], f32)
        xs_r = xs.rearrange("(t p) d -> p t d", p=128)
        for c in range(4):
            nc.sync.dma_start(out=x_sb[:, 4 * c:4 * c + 4, :], in_=xs_r[:, 4 * c:4 * c + 4, :])
        psA_cm = tc.tile_pool(name="psA", bufs=2, space="PSUM")
        psA = psA_cm.__enter__()

        def transpose_tile(t):
            xt_ps = psA.tile([D, 128], f32, tag="xtps", name=f"xtps{t}")
            nc.tensor.transpose(xt_ps, x_sb[:, t, :], idn)
            nc.scalar.copy(out=xT_all[:, t, :], in_=xt_ps)

        def q_tile(t):
            # Q' = x @ Wq'.T; bias row added during the PSUM->SBUF copy
            q_ps = psA.tile([128, D], f32, tag="qps", name=f"qps{t}", bufs=2)
            nc.tensor.matmul(q_ps, xT_all[:, t, :], wqt, start=True, stop=True)
            nc.vector.tensor_add(q_all[:, t * 128:(t + 1) * 128], q_ps, bqb)

        def kv_quad(qd):
            # K^T / V^T for 4 tiles in one 512-col matmul each
            xT4 = xT_all[:, 4 * qd:4 * qd + 4, :]
            nsl = slice(qd * 512, (qd + 1) * 512)
            k_ps = psA.tile([128, 512], f32, tag="kps", name=f"kps{qd}", bufs=2)
            v_ps = psA.tile([128, 512], f32, tag="vps", name=f"vps{qd}", bufs=2)
            nc.tensor.matmul(k_ps, wkt, xT4, start=True, stop=True)
            nc.tensor.matmul(v_ps, wvt, xT4, start=True, stop=True)
            nc.vector.tensor_scalar_add(kvt[:, 0, nsl], k_ps, biascol[:, 0:1])
            nc.vector.tensor_scalar_add(vt_b[:, nsl], v_ps, biascol[:, 1:2])

        for t in range(4):
            transpose_tile(t)
        for qd in range(4):
            for t in range(4 * qd, 4 * qd + 4):
                if t + 4 < NT:
                    transpose_tile(t + 4)
                q_tile(t)
            kv_quad(qd)
        psA_cm.__exit__(None, None, None)

        # ---- Phase B: m-major over all 4 column groups (4-way PE col-tiling) ----
        NG = 4
        psB_cm = tc.tile_pool(name="psB", bufs=1, space="PSUM")
        psB = psB_cm.__enter__()
        coef_ps = psB.tile([D, 2, NLOC], f32)
        for m in range(M):
            em = emp.tile([D, NLOC], evdt)
            nc.scalar.activation(out=em, in_=kvt[:, 0, :],
                                 func=mybir.ActivationFunctionType.Exp,
                                 scale=tms[:, m:m + 1])
            ev = evp.tile([D, NLOC], evdt)
            nc.vector.tensor_mul(ev, em, vt_b)
            for j in range(NG):
                sl = slice(j * 512, (j + 1) * 512)
                if m == 0:
                    nc.tensor.matmul(coef_ps[:, 0, sl], fmask[:, j, :], ev[:, sl],
                                     start=True, stop=False)
                else:
                    nc.tensor.matmul(coef_ps[32 * j:32 * j + 32, 0, sl], masks[:, m, :],
                                     ev[:, sl], start=False, stop=(m == M - 1),
                                     tile_position=(0, 32 * j))
            for j in range(NG):
                sl = slice(j * 512, (j + 1) * 512)
                if m == 0:
                    nc.tensor.matmul(coef_ps[:, 1, sl], fmask[:, 4 + j, :], em[:, sl],
                                     start=True, stop=False)
                else:
                    nc.tensor.matmul(coef_ps[32 * j:32 * j + 32, 1, sl], masks[:, m, :],
                                     em[:, sl], start=False, stop=(m == M - 1),
                                     tile_position=(0, 32 * j))

        # ---- Phase C: g = num/den, monomial coefficients, per-tile transpose ----
        for j in range(NG):
            nsl = slice(j * 512, (j + 1) * 512)
            nc.scalar.copy(out=coef_sb[:, :, nsl], in_=coef_ps[:, :, nsl])
        psB_cm.__exit__(None, None, None)
        psC = ctx.enter_context(tc.tile_pool(name="psC", bufs=2, space="PSUM"))
        psD = ctx.enter_context(tc.tile_pool(name="psD", bufs=2, space="PSUM"))
        for j in range(NG):
            nsl = slice(j * 512, (j + 1) * 512)
            nc.vector.reciprocal_approx_fast(out=rden[:, nsl], in_=coef_sb[:, 1, nsl])
            nc.vector.tensor_mul(g_sb[:, nsl], coef_sb[:, 0, nsl], rden[:, nsl])
            ct_ps = psC.tile([M, 512], f32, tag="ctps")
            nc.tensor.matmul(ct_ps, ainvt4[:, j, :], g_sb[:, nsl], start=True, stop=True)
            nc.scalar.copy(out=ct_sb[:, nsl], in_=ct_ps)
            for t in range(4 * j, 4 * j + 4):
                ctt_ps = psD.tile([128, M], f32, tag="cttps")
                nc.tensor.transpose(ctt_ps, ct_sb[:, t * 128:(t + 1) * 128], idn[0:M, 0:M])
                nc.scalar.copy(out=cts[:, t, :], in_=ctt_ps)

        # ---- Phase D: Horner, 4 tiles interleaved to hide DVE drains ----
        for q_ in range(NT // 4):
            ts_ = [4 * q_, 4 * q_ + 1, 4 * q_ + 2, 4 * q_ + 3]
            qs = [q_all[:, t * 128:(t + 1) * 128] for t in ts_]
            fbufs = []
            for i in range(4):
                fx0 = hor.tile([128, 128], f32, tag=f"f{i}0", name=f"f{i}0")
                fx1 = hor.tile([128, 128], f32, tag=f"f{i}1", name=f"f{i}1")
                fbufs.append([fx0, fx1])
            cur = [0, 0, 0, 0]
            for i, t in enumerate(ts_):
                nc.vector.tensor_scalar_mul(fbufs[i][0], qs[i], cts[:, t, M - 1:M])
            for k in range(M - 2, 0, -1):
                for i, t in enumerate(ts_):
                    nc.vector.scalar_tensor_tensor(out=fbufs[i][1 - cur[i]], in0=fbufs[i][cur[i]],
                                                   scalar=cts[:, t, k:k + 1], in1=qs[i],
                                                   op0=AluOpType.add, op1=AluOpType.mult)
                    cur[i] = 1 - cur[i]
            for i, t in enumerate(ts_):
                ox = outp.tile([128, 128], f32, tag=f"o{i}", name=f"o{i}")
                nc.vector.tensor_scalar_add(ox, fbufs[i][cur[i]], cts[:, t, 0:1])
                nc.sync.dma_start(out=OUT[t * 128:(t + 1) * 128, :], in_=ox)

    nc.finalize()
    _built["nc"] = nc
    return nc


def _host_prep(x, Wq, bq, Wk, bk, Wv, bv):
    """Fold positional encoding + scale into weights; build constants."""
    x = np.ascontiguousarray(x, dtype=np.float32)
    Wq = np.asarray(Wq, np.float32); bq = np.asarray(bq, np.float32)
    Wk = np.asarray(Wk, np.float32); bk = np.asarray(bk, np.float32)
    Wv = np.asarray(Wv, np.float32); bv = np.asarray(bv, np.float32)

    half = D // 2
    div = np.exp(np.arange(half, dtype=np.float64) * (-np.log(10000.0) / D))
    pe = np.zeros(D, np.float64)
    pe[0::2] = np.sin(np.arange(0, D, 2, dtype=np.float64) * div)
    pe[1::2] = np.cos(np.arange(1, D, 2, dtype=np.float64) * div)
    pe = pe.astype(np.float32)

    s = np.float32(1.0 / np.sqrt(D))
    Wq_s = (Wq * s).astype(np.float32)
    bq_s = (s * (bq + Wq @ pe)).astype(np.float32)
    bk_s = (bk + Wk @ pe).astype(np.float32)
    bv_s = (bv + Wv @ pe).astype(np.float32)

    # q' range for the Chebyshev interval
    Qp = x @ Wq_s.T + bq_s
    Tmax = float(np.abs(Qp).max()) * 1.0005

    theta = (2 * np.arange(M) + 1) * np.pi / (2 * M)
    tm = np.cos(theta) * Tmax                        # f64 Chebyshev points
    Vand = tm[:, None] ** np.arange(M)[None, :]
    Ainv = np.linalg.inv(Vand)                       # coeffs = Ainv @ g_samples

    masks = np.zeros((D, M, 32), np.float32)
    for mm in range(M):
        masks[:, mm, mm] = 1.0            # stream m -> in-group partition m
    fmask = np.zeros((8, D, D), np.float32)
    for j in range(4):
        fmask[j, :, 32 * j] = 1.0         # num m=0 -> partition 32j; other rows 0
        fmask[4 + j, :, :] = 1.0          # den m=0 -> every row gets a positive sum
        fmask[4 + j, :, 32 * j + 1:32 * j + M] = 0.0   # rows for m>=1 accumulate cleanly
    ainvt4 = np.zeros((4, D, M), np.float32)
    for j in range(4):
        ainvt4[j, 32 * j:32 * j + M, :] = Ainv.T.astype(np.float32)
    tms = np.tile(tm.astype(np.float32)[None, :], (D, 1))

    blob_parts = [
        np.ascontiguousarray(Wq_s.T),                               # WQT
        np.ascontiguousarray(Wk.T),                                 # WKT
        np.ascontiguousarray(Wv.T),                                 # WVT
        np.tile(bq_s[None, :], (D, 1)),                             # BQB
        np.eye(D, dtype=np.float32),                                # IDN
        fmask.transpose(1, 0, 2).reshape(D, 8 * D),                 # FMASK [p, i, c]
        ainvt4.transpose(1, 0, 2).reshape(D, 4 * M),                # AINVT4 [p, i, c]
        tms,                                                        # TMS
        masks.reshape(D, M * 32),                                   # MASKS [p, m, c]
        np.stack([bk_s, bv_s], axis=1),                             # BIASCOL
    ]
    blob = np.concatenate([p.astype(np.float32) for p in blob_parts], axis=1)
    # bias-row block: bq' on partition 0 (unused elsewhere)
    brow = np.zeros((D, D), np.float32)
    brow[0, :] = bq_s
    blob = np.concatenate([blob, brow], axis=1)
    consts = {"CONSTS": np.ascontiguousarray(blob)}
    return x, consts


def _run(inputs, trace=False):
    from concourse.bass_utils import run_bass_kernel_spmd
    x, consts = _host_prep(**inputs)
    nc = _build()
    in_maps = []
    for i in range(NCORES):
        m = {"xs": np.ascontiguousarray(x[i * NLOC:(i + 1) * NLOC])}
        m.update(consts)
        in_maps.append(m)
    res = run_bass_kernel_spmd(nc, in_maps, list(range(NCORES)), trace=trace)
    out = np.concatenate([r["out"] for r in res.results], axis=0)
    return out, res.exec_time_ns


def kernel(**inputs):
    out, _ = _run(inputs, trace=False)
    return out



# revision 8
# speedup vs baseline: 1.3060x; 1.3060x over previous
"""Trainium2 Bass kernel for per-node rank-1 self-attention (NodeFeatureSelfAttention).

Math: for each node n (row of x):
    q = s*(Wq @ xp + bq); k = Wk @ xp + bk; v = Wv @ xp + bv   (xp = x + pe)
    out[i] = sum_j softmax_j(q_i * k_j)[j] * v_j = g(q_i)
with g(t) = sum_j exp(t*k_j)*v_j / sum_j exp(t*k_j), a smooth scalar function
per node. We sample g at M shared Chebyshev points t_m (ACT engine exps),
reduce with masked-ones matmuls on the PE, convert samples -> monomial
coefficients with a shared M x M matrix (PE), and evaluate the degree-(M-1)
interpolant per element with Horner steps split across the DVE (bf16
scalar_tensor_tensor) and GPSIMD (f32 broadcast tensor_tensor) engines.

Data-parallel over nodes across 8 NeuronCores; weights replicated.
"""
import sys
sys.path.insert(0, "/opt/trn_rl_repo")
import numpy as np
from contextlib import ExitStack

N, D = 16384, 128
NCORES = 8
NLOC = N // NCORES            # 2048 nodes per core
NT = NLOC // 128              # 16 node-tiles per core
M = 5                         # Chebyshev sample count (degree M-1 interpolant)
NT_GP = 3                     # leading tiles evaluated on GPSIMD (f32)

_built = {}


def _build():
    """Build + finalize the (data-independent) bass module once."""
    if "nc" in _built:
        return _built["nc"]
    import concourse.bacc as bacc
    import concourse.tile as tile
    from concourse import mybir

    f32 = mybir.dt.float32
    bf16 = mybir.dt.bfloat16
    nc = bacc.Bacc()

    xs = nc.declare_dram_parameter("xs", [NLOC, D], f32, isOutput=False)
    # f32 consts: IDN D | AINVT4 4M | TMS M | BIASCOL 2
    NC32 = D + 4 * M + M + 2
    CONSTS = nc.declare_dram_parameter("CONSTS", [D, NC32], f32, isOutput=False)
    # bf16 consts: WQT D | WKT D | WVT D | FMASK 8D | MASKS 32M | BQROW D
    NCB = 3 * D + 8 * D + 32 * M + D
    CONSTB = nc.declare_dram_parameter("CONSTB", [D, NCB], bf16, isOutput=False)
    OUT = nc.declare_dram_parameter("out", [NLOC, D], f32, isOutput=True)

    with tile.TileContext(nc) as tc, ExitStack() as ctx:
        from concourse.mybir import AluOpType
        singles = ctx.enter_context(tc.tile_pool(name="singles", bufs=1))
        emp = ctx.enter_context(tc.tile_pool(name="emp", bufs=3))
        evp = ctx.enter_context(tc.tile_pool(name="evp", bufs=3))
        hor = ctx.enter_context(tc.tile_pool(name="hor", bufs=4))
        outp = ctx.enter_context(tc.tile_pool(name="outp", bufs=6))

        # ---- constants ----
        cblob = singles.tile([D, NC32], f32)
        nc.sync.dma_start(out=cblob[:, :], in_=CONSTS[:, :])
        bblob = singles.tile([D, NCB], bf16)
        nc.sync.dma_start(out=bblob[:, 0:4 * D], in_=CONSTB[:, 0:4 * D])
        nc.sync.dma_start(out=bblob[:, 4 * D:NCB], in_=CONSTB[:, 4 * D:NCB])
        o = 0
        idn = cblob[:, o:o + D]; o += D
        ainvt4 = cblob[:, o:o + 4 * M].rearrange("p (i c) -> p i c", i=4); o += 4 * M
        tms = cblob[:, o:o + M]; o += M
        biascol = cblob[:, o:o + 2]  # [bk', bv'] per-partition columns
        o += 2
        ob = 0
        wqt = bblob[:, ob:ob + D]; ob += D
        wkt = bblob[:, ob:ob + D]; ob += D
        wvt = bblob[:, ob:ob + D]; ob += D
        fmask = bblob[:, ob:ob + 8 * D].rearrange("p (i c) -> p i c", i=8); ob += 8 * D
        masks = bblob[:, ob:ob + 32 * M].rearrange("p (i c) -> p i c", i=M); ob += 32 * M
        bqrow = bblob[0:1, ob:ob + D]   # bq' on partition 0
        ob += D
        ones = singles.tile([1, 128], bf16)
        nc.vector.memset(ones[:, :], 1.0)

        xT_all = singles.tile([D, NT, 128], bf16)     # x^T per tile (bf16)
        q_bf = singles.tile([D, NT, 128], bf16)       # Q' tiles (DVE Horner), t>=NT_GP
        q_f3 = singles.tile([D, NT_GP, 128], f32)     # Q' tiles (GPSIMD Horner)
        kvt = singles.tile([D, NLOC], bf16)           # K^T [j, n]
        vt_b = singles.tile([D, NLOC], bf16)          # V^T
        cts = singles.tile([D, NT, M], f32)           # per-tile monomial coeffs
        coef_sb = singles.tile([D, 2, NLOC], f32)     # [p, {num,den}, n]
        rden = singles.tile([D, NLOC], f32)
        g_sb = singles.tile([D, NLOC], f32)
        ct_sb = singles.tile([M, NLOC], f32)

        # ---- Phase A: load x (4 DMAs), transpose + QKV per quad ----
        x_sb = singles.tile([D, NT, D], f32)
        xs_r = xs.rearrange("(t p) d -> p t d", p=128)
        for c in range(4):
            nc.sync.dma_start(out=x_sb[:, 4 * c:4 * c + 4, :], in_=xs_r[:, 4 * c:4 * c + 4, :])
        psA_cm = tc.tile_pool(name="psA", bufs=2, space="PSUM")
        psA = psA_cm.__enter__()

        def q_pair(t0):
            # Q' data matmuls + shared-weight rank-1 bias matmuls for 2 tiles
            qp = []
            for t in (t0, t0 + 1):
                q_ps = psA.tile([128, D], f32, tag=f"qps{t % 2}", name=f"qps{t}", bufs=1)
                nc.tensor.matmul(q_ps, xT_all[:, t, :], wqt, start=True, stop=False)
                qp.append(q_ps)
            for i, t in enumerate((t0, t0 + 1)):
                nc.tensor.matmul(qp[i], ones, bqrow, start=False, stop=True)
            for i, t in enumerate((t0, t0 + 1)):
                if t < NT_GP:
                    nc.scalar.copy(out=q_f3[:, t, :], in_=qp[i])
                else:
                    nc.scalar.copy(out=q_bf[:, t, :], in_=qp[i])

        for qd in range(4):
            # 4 transposes (2 PSUM buffers, ACT copy trails)
            for t in range(4 * qd, 4 * qd + 4):
                xt_ps = psA.tile([D, 128], f32, tag=f"xtps{t % 2}", name=f"xtps{t}", bufs=1)
                nc.tensor.transpose(xt_ps, x_sb[:, t, :], idn)
                nc.scalar.copy(out=xT_all[:, t, :], in_=xt_ps)
            # K^T / V^T for the quad: one 512-col matmul each, bias in ACT copy
            xT4 = xT_all[:, 4 * qd:4 * qd + 4, :]
            nsl = slice(qd * 512, (qd + 1) * 512)
            k_ps = psA.tile([128, 512], f32, tag="kps", name=f"kps{qd}", bufs=2)
            v_ps = psA.tile([128, 512], f32, tag="vps", name=f"vps{qd}", bufs=2)
            nc.tensor.matmul(k_ps, wkt, xT4, start=True, stop=True)
            nc.tensor.matmul(v_ps, wvt, xT4, start=True, stop=True)
            nc.scalar.add(out=kvt[:, nsl], in_=k_ps, add=biascol[:, 0:1])
            nc.scalar.add(out=vt_b[:, nsl], in_=v_ps, add=biascol[:, 1:2])
            q_pair(4 * qd)
            q_pair(4 * qd + 2)
        psA_cm.__exit__(None, None, None)

        # ---- Phase B: m-major over all 4 column groups (4-way PE col-tiling) ----
        NG = 4
        psB_cm = tc.tile_pool(name="psB", bufs=1, space="PSUM")
        psB = psB_cm.__enter__()
        coef_ps = psB.tile([D, 2, NLOC], f32)
        for m in range(M):
            em = emp.tile([D, NLOC], bf16)
            nc.scalar.activation(out=em, in_=kvt,
                                 func=mybir.ActivationFunctionType.Exp,
                                 scale=tms[:, m:m + 1])
            ev = evp.tile([D, NLOC], bf16)
            nc.vector.tensor_mul(ev, em, vt_b)
            for j in range(NG):
                sl = slice(j * 512, (j + 1) * 512)
                if m == 0:
                    nc.tensor.matmul(coef_ps[:, 0, sl], fmask[:, j, :], ev[:, sl],
                                     start=True, stop=False)
                else:
                    nc.tensor.matmul(coef_ps[32 * j:32 * j + 32, 0, sl], masks[:, m, :],
                                     ev[:, sl], start=False, stop=(m == M - 1),
                                     tile_position=(0, 32 * j))
            for j in range(NG):
                sl = slice(j * 512, (j + 1) * 512)
                if m == 0:
                    nc.tensor.matmul(coef_ps[:, 1, sl], fmask[:, 4 + j, :], em[:, sl],
                                     start=True, stop=False)
                else:
                    nc.tensor.matmul(coef_ps[32 * j:32 * j + 32, 1, sl], masks[:, m, :],
                                     em[:, sl], start=False, stop=(m == M - 1),
                                     tile_position=(0, 32 * j))

        # ---- Phase C: g = num/den, monomial coefficients, per-tile transpose ----
        for j in range(NG):
            nsl = slice(j * 512, (j + 1) * 512)
            nc.scalar.copy(out=coef_sb[:, :, nsl], in_=coef_ps[:, :, nsl])
        psB_cm.__exit__(None, None, None)
        psC = ctx.enter_context(tc.tile_pool(name="psC", bufs=2, space="PSUM"))
        psD = ctx.enter_context(tc.tile_pool(name="psD", bufs=2, space="PSUM"))
        for j in range(NG):
            nsl = slice(j * 512, (j + 1) * 512)
            nc.vector.reciprocal_approx_fast(out=rden[:, nsl], in_=coef_sb[:, 1, nsl])
            if j < 2:
                nc.gpsimd.tensor_tensor(out=g_sb[:, nsl], in0=coef_sb[:, 0, nsl],
                                        in1=rden[:, nsl], op=AluOpType.mult)
            else:
                nc.vector.tensor_mul(g_sb[:, nsl], coef_sb[:, 0, nsl], rden[:, nsl])
            ct_ps = psC.tile([M, 512], f32, tag="ctps")
            nc.tensor.matmul(ct_ps, ainvt4[:, j, :], g_sb[:, nsl], start=True, stop=True)
            nc.scalar.copy(out=ct_sb[:, nsl], in_=ct_ps)
            for t in range(4 * j, 4 * j + 4):
                ctt_ps = psD.tile([128, M], f32, tag="cttps")
                nc.tensor.transpose(ctt_ps, ct_sb[:, t * 128:(t + 1) * 128], idn[0:M, 0:M])
                nc.scalar.copy(out=cts[:, t, :], in_=ctt_ps)

        # ---- Phase D: Horner on DVE (bf16 STT) + GPSIMD (f32 broadcast TT) ----
        def horner_gp(t):
            q = q_f3[:, t, :]
            f0 = hor.tile([128, 128], f32, tag=f"g{t}a", name=f"g{t}a")
            f1 = hor.tile([128, 128], f32, tag=f"g{t}b", name=f"g{t}b")
            cb = [cts[:, t, k:k + 1].broadcast_to([128, 128]) for k in range(M)]
            nc.gpsimd.tensor_tensor(out=f0, in0=q, in1=cb[M - 1], op=AluOpType.mult)
            for k in range(M - 2, 0, -1):
                nc.gpsimd.tensor_tensor(out=f1, in0=f0, in1=cb[k], op=AluOpType.add)
                nc.gpsimd.tensor_tensor(out=f0, in0=f1, in1=q, op=AluOpType.mult)
            ox = outp.tile([128, 128], f32, tag=f"og{t}", name=f"og{t}")
            nc.gpsimd.tensor_tensor(out=ox, in0=f0, in1=cb[0], op=AluOpType.add)
            nc.sync.dma_start(out=OUT[t * 128:(t + 1) * 128, :], in_=ox)

        def horner_dve_group(ts_):
            qs = [q_bf[:, t, :] for t in ts_]
            fbufs = []
            for i, t in enumerate(ts_):
                fx0 = hor.tile([128, 128], mybir.dt.bfloat16, tag=f"f{i}0", name=f"f{t}0")
                fx1 = hor.tile([128, 128], mybir.dt.bfloat16, tag=f"f{i}1", name=f"f{t}1")
                fbufs.append([fx0, fx1])
            cur = [0] * len(ts_)
            for i, t in enumerate(ts_):
                nc.vector.tensor_scalar_mul(fbufs[i][0], qs[i], cts[:, t, M - 1:M])
            for k in range(M - 2, 0, -1):
                for i, t in enumerate(ts_):
                    nc.vector.scalar_tensor_tensor(out=fbufs[i][1 - cur[i]], in0=fbufs[i][cur[i]],
                                                   scalar=cts[:, t, k:k + 1], in1=qs[i],
                                                   op0=AluOpType.add, op1=AluOpType.mult)
                    cur[i] = 1 - cur[i]
            for i, t in enumerate(ts_):
                ox = outp.tile([128, 128], f32, tag=f"o{i}", name=f"o{t}")
                nc.vector.tensor_scalar_add(ox, fbufs[i][cur[i]], cts[:, t, 0:1])
                nc.sync.dma_start(out=OUT[t * 128:(t + 1) * 128, :], in_=ox)

        for t in range(NT_GP):
            horner_gp(t)
        dve_tiles = list(range(NT_GP, NT))
        for g0 in range(0, len(dve_tiles), 4):
            horner_dve_group(dve_tiles[g0:g0 + 4])

    nc.finalize()
    _built["nc"] = nc
    return nc


def _host_prep(x, Wq, bq, Wk, bk, Wv, bv):
    """Fold positional encoding + scale into weights; build constants."""
    x = np.ascontiguousarray(x, dtype=np.float32)
    Wq = np.asarray(Wq, np.float32); bq = np.asarray(bq, np.float32)
    Wk = np.asarray(Wk, np.float32); bk = np.asarray(bk, np.float32)
    Wv = np.asarray(Wv, np.float32); bv = np.asarray(bv, np.float32)

    half = D // 2
    div = np.exp(np.arange(half, dtype=np.float64) * (-np.log(10000.0) / D))
    pe = np.zeros(D, np.float64)
    pe[0::2] = np.sin(np.arange(0, D, 2, dtype=np.float64) * div)
    pe[1::2] = np.cos(np.arange(1, D, 2, dtype=np.float64) * div)
    pe = pe.astype(np.float32)

    s = np.float32(1.0 / np.sqrt(D))
    Wq_s = (Wq * s).astype(np.float32)
    bq_s = (s * (bq + Wq @ pe)).astype(np.float32)
    bk_s = (bk + Wk @ pe).astype(np.float32)
    bv_s = (bv + Wv @ pe).astype(np.float32)

    # q' range for the Chebyshev interval
    Qp = x @ Wq_s.T + bq_s
    Tmax = float(np.abs(Qp).max()) * 1.0005

    theta = (2 * np.arange(M) + 1) * np.pi / (2 * M)
    tm = np.cos(theta) * Tmax                        # f64 Chebyshev points
    Vand = tm[:, None] ** np.arange(M)[None, :]
    Ainv = np.linalg.inv(Vand)                       # coeffs = Ainv @ g_samples

    masks = np.zeros((D, M, 32), np.float32)
    for mm in range(M):
        masks[:, mm, mm] = 1.0            # stream m -> in-group partition m
    fmask = np.zeros((8, D, D), np.float32)
    for j in range(4):
        fmask[j, :, 32 * j] = 1.0         # num m=0 -> partition 32j; other rows 0
        fmask[4 + j, :, :] = 1.0          # den m=0 -> every row gets a positive sum
        fmask[4 + j, :, 32 * j + 1:32 * j + M] = 0.0   # rows for m>=1 accumulate cleanly
    ainvt4 = np.zeros((4, D, M), np.float32)
    for j in range(4):
        ainvt4[j, 32 * j:32 * j + M, :] = Ainv.T.astype(np.float32)
    tms = np.tile(tm.astype(np.float32)[None, :], (D, 1))

    blob32 = np.concatenate([
        np.eye(D, dtype=np.float32),                                # IDN
        ainvt4.transpose(1, 0, 2).reshape(D, 4 * M),                # AINVT4
        tms,                                                        # TMS
        np.stack([bk_s, bv_s], axis=1),                             # BIASCOL
    ], axis=1).astype(np.float32)

    bqrow = np.zeros((D, D), np.float32)
    bqrow[0, :] = bq_s

    import ml_dtypes
    blobb = np.concatenate([
        np.ascontiguousarray(Wq_s.T),                               # WQT
        np.ascontiguousarray(Wk.T),                                 # WKT
        np.ascontiguousarray(Wv.T),                                 # WVT
        fmask.transpose(1, 0, 2).reshape(D, 8 * D),                 # FMASK
        masks.reshape(D, M * 32),                                   # MASKS
        bqrow,                                                      # BQROW
    ], axis=1).astype(ml_dtypes.bfloat16)

    consts = {"CONSTS": np.ascontiguousarray(blob32),
              "CONSTB": np.ascontiguousarray(blobb)}
    return x, consts


def _run(inputs, trace=False):
    from concourse.bass_utils import run_bass_kernel_spmd
    x, consts = _host_prep(**inputs)
    nc = _build()
    in_maps = []
    for i in range(NCORES):
        m = {"xs": np.ascontiguousarray(x[i * NLOC:(i + 1) * NLOC])}
        m.update(consts)
        in_maps.append(m)
    res = run_bass_kernel_spmd(nc, in_maps, list(range(NCORES)), trace=trace)
    out = np.concatenate([r["out"] for r in res.results], axis=0)
    return out, res.exec_time_ns


def kernel(**inputs):
    out, _ = _run(inputs, trace=False)
    return out
